# revision 25
# baseline (speedup 1.0000x reference)
"""MeshGaussiansField forward kernel for 8 Trainium2 NeuronCores.

Strategy (data-parallel over faces, per the sharding hint):
  - faces sharded 8 ways (62500/core, padded to a tile multiple);
  - vertices + all MLP weights replicated to every core;
  - per-core Bass kernel: indirect-DMA gather of face vertices, face-major
    geometry (centroid/normal/view), transposed-activation f32r MLP on the
    tensor engine (heads computed back in face-major via small matmuls),
    quaternion/covariance math face-major, one output DMA per tile;
  - host only pads/shards/concatenates and folds weight-weight products
    (geo_w1[:,1:] @ rw0[9:] -> one 256x256 matrix, a weights-only fold).
"""
import sys
import numpy as np

sys.path.insert(0, '/opt/trn_rl_repo')

import concourse.bass as bass
import concourse.bacc as bacc
import concourse.tile as tile
import concourse.mybir as mybir
from concourse.bass_utils import run_bass_kernel_spmd
from concourse.masks import make_identity

F32 = mybir.dt.float32
F32R = mybir.dt.float32r
I32 = mybir.dt.int32
AF = mybir.ActivationFunctionType
ALU = mybir.AluOpType

N_CORES = 8
V = 250000
F_TOTAL = 500000
F_CORE = F_TOTAL // N_CORES          # 62500
TILE_N = 1024                        # faces per macro tile
T = TILE_N // 128                    # 8 faces per partition per tile
NB = TILE_N // 512                   # MLP N-blocks per tile
N_TILES = (F_CORE + TILE_N - 1) // TILE_N
F_PAD = N_TILES * TILE_N
DH = 256
C0 = 0.28209479177387814
PI = float(np.pi)


def _fit_trig_coefs():
    """Polynomials in w = u^2 for u in [-pi/2, pi/2]:
    cos(u) ~ C(w);  sin(u) ~ u * S(w).  Degree 4 each (even/odd series)."""
    u = np.linspace(-np.pi / 2, np.pi / 2, 20001)
    w = u * u
    cc = np.polynomial.polynomial.polyfit(w, np.cos(u), 4)
    ss = np.polynomial.polynomial.polyfit(w, np.sinc(u / np.pi), 4)
    assert np.abs(np.polynomial.polynomial.polyval(w, cc) - np.cos(u)).max() < 1e-6
    assert np.abs(u * np.polynomial.polynomial.polyval(w, ss) - np.sin(u)).max() < 1e-6
    return [float(x) for x in cc], [float(x) for x in ss]


COS_C, SIN_C = _fit_trig_coefs()

SKIP = set()  # debug knobs: subsets of {"gather", "geom", "mlp", "fin"}

_CACHE = {}


def _build_program():
    nc = bacc.Bacc("TRN2", target_bir_lowering=False, debug=False,
                   num_devices=N_CORES)

    def din(name, shape, dt=F32):
        return nc.dram_tensor(name, shape, dt, kind="ExternalInput").ap()

    faces_ap = din("faces", [F_PAD, 3], I32)
    verts_ap = din("verts", [V, 3])
    cam_ap = din("cam", [1, 3])
    gw0_ap = din("gw0", [3, DH])
    gb0_ap = din("gb0", [DH])
    wc_ap = din("wc", [DH, DH])
    rgeom_ap = din("rgeom", [9, DH])
    rb0e_ap = din("rb0e", [DH])
    rw1_ap = din("rw1", [DH, DH])
    rb1_ap = din("rb1", [DH])
    rw2_ap = din("rw2", [DH, DH])
    rb2_ap = din("rb2", [DH])
    rw3_ap = din("rw3", [DH, DH])
    rb3_ap = din("rb3", [DH])
    hw8_ap = din("hw8", [DH, 8])        # [rw4 | sw | aw | 0] (col 7 zero)
    hb10_ap = din("hb10", [1, 10])      # [rb4, sb, ab, 0, geo_b1[0], 0]
    wo_ap = din("wo", [DH, 2])          # [geo_w1[:, :1] | 0]
    out_ap = nc.dram_tensor("out", [F_PAD, 23], F32, kind="ExternalOutput").ap()

    with tile.TileContext(nc) as tc:
        wpool = tc.alloc_tile_pool(name="weights", bufs=1)
        spool = tc.alloc_tile_pool(name="acts", bufs=3)
        fpool = tc.alloc_tile_pool(name="facemajor", bufs=2)
        ppool = tc.alloc_tile_pool(name="psum", bufs=6, space="PSUM")
        ppoolh = tc.alloc_tile_pool(name="psumh", bufs=1, space="PSUM")
        ppool2 = tc.alloc_tile_pool(name="psum2", bufs=1, space="PSUM")

        # ---------------- one-time setup ----------------
        ident = wpool.tile([128, 128], F32)
        make_identity(nc, ident[:])

        def load_round(name, ap, p, f):
            raw = wpool.tile([p, f], F32, tag=f"{name}_raw")
            nc.sync.dma_start(raw[:], ap)
            w = wpool.tile([p, f], F32R, tag=name)
            nc.vector.tensor_copy(w[:], raw[:])
            return w

        gw0 = load_round("gw0", gw0_ap[:], 3, DH)
        rgeom = load_round("rgeom", rgeom_ap[:], 9, DH)
        wck = [load_round(f"wc{k}", wc_ap[k * 128:(k + 1) * 128, :], 128, DH)
               for k in range(2)]
        rwk = [[load_round(f"rw{li}{k}", ap[k * 128:(k + 1) * 128, :], 128, DH)
                for k in range(2)]
               for li, ap in enumerate([rw1_ap, rw2_ap, rw3_ap])]
        hwk = [load_round(f"hw{k}", hw8_ap[k * 128:(k + 1) * 128, :], 128, 8)
               for k in range(2)]
        wok = [load_round(f"wo{k}", wo_ap[k * 128:(k + 1) * 128, :], 128, 2)
               for k in range(2)]

        def load_bias(name, ap, n):
            b = wpool.tile([n, 1], F32, tag=name)
            nc.sync.dma_start(b[:], ap[:, None])
            return b

        gb0 = [load_bias(f"gb0{k}", gb0_ap[k * 128:(k + 1) * 128], 128) for k in range(2)]
        rb0e = [load_bias(f"rb0e{k}", rb0e_ap[k * 128:(k + 1) * 128], 128) for k in range(2)]
        rbs = [[load_bias(f"rb{li}{k}", ap[k * 128:(k + 1) * 128], 128) for k in range(2)]
               for li, ap in enumerate([rb1_ap, rb2_ap, rb3_ap])]

        # broadcast camera and head-bias to all 128 partitions via ones outer
        ones_col = wpool.tile([1, 128], F32)
        nc.gpsimd.memset(ones_col[:], 1.0)

        def bcast_row(name, ap, f):
            row = wpool.tile([1, f], F32, tag=f"{name}_row")
            nc.sync.dma_start(row[:], ap)
            ps = ppool2.tile([128, f], F32, space="PSUM", tag="trans")
            nc.tensor.matmul(ps[:], ones_col[:], row[:], start=True, stop=True)
            full = wpool.tile([128, f], F32, tag=name)
            nc.vector.tensor_copy(full[:], ps[:])
            return full

        cam_fm = bcast_row("cam_fm", cam_ap[:], 3)
        hb_fm = bcast_row("hb_fm", hb10_ap[:], 10)

        # ---------------- per-tile body ----------------
        for t_i in range(N_TILES):
            base = t_i * TILE_N

            fs = fpool.tile([128, 3 * T], I32, tag="fs")
            nc.sync.dma_start(
                fs[:], faces_ap[base:base + TILE_N, :].rearrange(
                    "(p j) c -> p (j c)", p=128))

            # ---- gather v0/v1/v2 as [128, T, 3] face-major tiles ----
            vms = []
            for c in range(3):
                vm = fpool.tile([128, T, 3], F32, tag=f"vm{c}")
                if "gather" not in SKIP:
                    for j in range(T):
                        nc.gpsimd.indirect_dma_start(
                            out=vm[:, j, :], out_offset=None, in_=verts_ap[:],
                            in_offset=bass.IndirectOffsetOnAxis(
                                ap=fs[:, 3 * j + c:3 * j + c + 1], axis=0))
                else:
                    nc.vector.memset(vm[:], 0.25 * (c + 1))
                vms.append(vm)
            v0, v1, v2 = vms

            # ---- geometry (face-major) ----
            geom = fpool.tile([128, T, 9], F32, tag="geom")
            xyz = geom[:, :, 0:3]
            view = geom[:, :, 3:6]
            nrm = geom[:, :, 6:9]

            if "geom" in SKIP:
                nc.vector.memset(geom[:], 0.5)
            else:
                tmp3 = fpool.tile([128, T, 3], F32, tag="tmp3")
                nc.vector.tensor_add(tmp3[:], v0[:], v1[:])
                nc.vector.tensor_add(tmp3[:], tmp3[:], v2[:])
                nc.vector.tensor_scalar_mul(xyz, tmp3[:], 1.0 / 3.0)

                e1 = fpool.tile([128, T, 3], F32, tag="e1")
                nc.vector.tensor_sub(e1[:], v0[:], v1[:])
                e2 = fpool.tile([128, T, 3], F32, tag="e2")
                nc.vector.tensor_sub(e2[:], v0[:], v2[:])

                cr = fpool.tile([128, T, 3], F32, tag="cr")
                prod = fpool.tile([128, T, 1], F32, tag="prod")
                for a in range(3):
                    b_, c_ = (a + 1) % 3, (a + 2) % 3
                    nc.vector.tensor_mul(cr[:, :, a:a + 1], e1[:, :, b_:b_ + 1],
                                         e2[:, :, c_:c_ + 1])
                    nc.vector.tensor_mul(prod[:], e1[:, :, c_:c_ + 1],
                                         e2[:, :, b_:b_ + 1])
                    nc.vector.tensor_sub(cr[:, :, a:a + 1], cr[:, :, a:a + 1], prod[:])

                def normalize(dst, src, tagp):
                    # 1/||src|| = exp(-0.5*ln(max(sum(src^2),1e-24)))
                    sq = fpool.tile([128, T, 3], F32, tag=f"{tagp}_sq")
                    nc.vector.tensor_mul(sq[:], src, src)
                    ss = fpool.tile([128, T], F32, tag=f"{tagp}_ss")
                    nc.vector.reduce_sum(ss[:], sq[:], axis=mybir.AxisListType.X)
                    nc.vector.tensor_scalar_max(ss[:], ss[:], 1e-24)
                    lg = fpool.tile([128, T], F32, tag=f"{tagp}_l")
                    nc.scalar.activation(lg[:], ss[:], AF.Ln)
                    rinv = fpool.tile([128, T], F32, tag=f"{tagp}_r")
                    nc.scalar.activation(rinv[:], lg[:], AF.Exp, scale=-0.5)
                    nc.vector.tensor_mul(dst, src,
                                         rinv[:, :, None].to_broadcast([128, T, 3]))

                normalize(nrm, cr[:], "nn")
                dvec = fpool.tile([128, T, 3], F32, tag="dvec")
                nc.vector.tensor_sub(dvec[:], xyz,
                                     cam_fm[:, None, :].to_broadcast([128, T, 3]))
                normalize(view, dvec[:], "vv")

            # ---- MLP (transposed acts, f32r) ----
            scr8 = fpool.tile([128, T, 10], F32, tag="scr8")
            if "mlp" in SKIP:
                nc.vector.memset(scr8[:], 0.125)
            else:
                geomT = spool.tile([9, TILE_N], F32R, tag="geomT")
                for j in range(T):
                    tps = ppool2.tile([9, 128], F32, space="PSUM", tag="trans")
                    nc.tensor.transpose(tps[:], geom[:, j, :], ident[:])
                    nc.scalar.copy(geomT[:, j * 128:(j + 1) * 128], tps[:])

                ghTs, hs = [], []
                for nb in range(NB):
                    sl = slice(nb * 512, (nb + 1) * 512)

                    def mm_layer(lhsT_chunks, rhs_chunks, m_out, n=512):
                        outs = []
                        for m in range(m_out):
                            ps = ppool.tile([128, n], F32, space="PSUM", tag="mm")
                            for ki, (lh, rh) in enumerate(zip(lhsT_chunks, rhs_chunks)):
                                msl = (lh[:, m * 128:(m + 1) * 128]
                                       if m_out > 1 else lh[:])
                                nc.tensor.matmul(ps[:], msl, rh, start=(ki == 0),
                                                 stop=(ki == len(lhsT_chunks) - 1))
                            outs.append(ps)
                        return outs

                    # gh = softplus(z) = ln(1 + exp(z))
                    gh_ps = mm_layer([gw0], [geomT[0:3, sl]], 2)
                    ghT = []
                    for m in range(2):
                        ez = spool.tile([128, 512], F32, tag=f"ez{m}")
                        nc.scalar.activation(ez[:], gh_ps[m][:], AF.Exp,
                                             bias=gb0[m][:])
                        g = spool.tile([128, 512], F32R, tag=f"ghT{nb}{m}")
                        nc.scalar.activation(g[:], ez[:], AF.Ln, bias=1.0)
                        ghT.append(g)

                    # h1 = relu(wc^T gh + rgeom^T geom + rb0e)
                    h = []
                    for m in range(2):
                        ps = ppool.tile([128, 512], F32, space="PSUM", tag="mm")
                        nc.tensor.matmul(ps[:], wck[0][:, m * 128:(m + 1) * 128],
                                         ghT[0][:], start=True, stop=False)
                        nc.tensor.matmul(ps[:], wck[1][:, m * 128:(m + 1) * 128],
                                         ghT[1][:], start=False, stop=False)
                        nc.tensor.matmul(ps[:], rgeom[:, m * 128:(m + 1) * 128],
                                         geomT[:, sl], start=False, stop=True)
                        hh = spool.tile([128, 512], F32R, tag=f"h1_{m}")
                        nc.scalar.activation(hh[:], ps[:], AF.Relu, bias=rb0e[m][:])
                        h.append(hh)

                    for li in range(3):
                        ps2 = mm_layer(rwk[li], [h[0][:], h[1][:]], 2)
                        hn = []
                        for m in range(2):
                            tg = (f"r{li}{nb}_{m}" if li == 2 else f"r{li}_{m}")
                            hh = spool.tile([128, 512], F32R, tag=tg)
                            nc.scalar.activation(hh[:], ps2[m][:], AF.Relu,
                                                 bias=rbs[li][m][:])
                            hn.append(hh)
                        h = hn

                    ghTs.append(ghT)
                    hs.append(h)

                # ---- heads, directly face-major: [128f, 10] per subtile j ----
                hfm = ppoolh.tile([128, T, 10], F32, space="PSUM", tag="hfm")
                for j in range(T):
                    nb, jj = divmod(j, 4)
                    fsl = slice(jj * 128, (jj + 1) * 128)
                    for ki in range(2):
                        nc.tensor.matmul(hfm[:, j, 0:8], hs[nb][ki][:, fsl],
                                         hwk[ki][:], start=(ki == 0),
                                         stop=(ki == 1))
                    for ki in range(2):
                        nc.tensor.matmul(hfm[:, j, 8:10], ghTs[nb][ki][:, fsl],
                                         wok[ki][:], start=(ki == 0),
                                         stop=(ki == 1))
                # bias add (bias varies along free dim, same per partition)
                nc.vector.tensor_add(scr8[:], hfm[:],
                                     hb_fm[:, None, :].to_broadcast([128, T, 10]))

            # ---- face-major finale -> out_tile [128, T, 23] ----
            ot = fpool.tile([128, T, 23], F32, tag="ot")
            if "fin" in SKIP:
                nc.vector.tensor_copy(ot[:, :, 0:10], scr8[:])
                nc.vector.tensor_copy(ot[:, :, 10:20], scr8[:])
                nc.vector.tensor_copy(ot[:, :, 20:23], scr8[:, :, 0:3])
            else:
                nc.vector.tensor_copy(ot[:, :, 0:3], xyz)
                nc.vector.tensor_copy(ot[:, :, 3:6], nrm)

                # sigmoids for color/scale/theta via 1/(1+exp(-x))
                esig = fpool.tile([128, T, 7], F32, tag="esig")
                nc.scalar.activation(esig[:], scr8[:, :, 0:7], AF.Exp, scale=-1.0)
                nc.vector.tensor_scalar_add(esig[:], esig[:], 1.0)
                sigm = fpool.tile([128, T, 7], F32, tag="sigm")
                nc.vector.reciprocal(sigm[:], esig[:])

                # features_dc = (sigmoid(colorpre) - 0.5) / C0
                nc.vector.tensor_scalar(ot[:, :, 6:9], sigm[:, :, 0:3], 1.0 / C0,
                                        -0.5 / C0, ALU.mult, ALU.add)

                # scale = sigmoid(scalepre); scaling_log = ln(scale)
                scl = sigm[:, :, 3:6]
                nc.scalar.activation(ot[:, :, 9:12], scl, AF.Ln)

                # theta: u = pi*sigmoid(thetapre) - pi/2
                # quat_w = cos(half) = -sin(u); sin(half) = cos(u)
                tsig = sigm[:, :, 6:7]
                uu = fpool.tile([128, T, 1], F32, tag="uu")
                nc.vector.tensor_scalar(uu[:], tsig, PI, -PI / 2.0, ALU.mult, ALU.add)
                u2 = fpool.tile([128, T, 1], F32, tag="u2")
                nc.vector.tensor_mul(u2[:], uu[:], uu[:])
                p2 = fpool.tile([128, T, 1], F32, tag="p2")
                nc.vector.tensor_mul(p2[:], u2[:], u2[:])
                p3 = fpool.tile([128, T, 1], F32, tag="p3")
                nc.vector.tensor_mul(p3[:], p2[:], u2[:])
                p4 = fpool.tile([128, T, 1], F32, tag="p4")
                nc.vector.tensor_mul(p4[:], p2[:], p2[:])

                cosu = fpool.tile([128, T, 1], F32, tag="cosu")
                nc.vector.tensor_scalar(cosu[:], u2[:], COS_C[1], COS_C[0],
                                        ALU.mult, ALU.add)
                for pw, cf in ((p2, COS_C[2]), (p3, COS_C[3]), (p4, COS_C[4])):
                    nc.vector.scalar_tensor_tensor(cosu[:], pw[:], cf, cosu[:],
                                                   ALU.mult, ALU.add)
                spoly = fpool.tile([128, T, 1], F32, tag="spoly")
                nc.vector.tensor_scalar(spoly[:], u2[:], SIN_C[1], SIN_C[0],
                                        ALU.mult, ALU.add)
                for pw, cf in ((p2, SIN_C[2]), (p3, SIN_C[3]), (p4, SIN_C[4])):
                    nc.vector.scalar_tensor_tensor(spoly[:], pw[:], cf, spoly[:],
                                                   ALU.mult, ALU.add)
                negu = fpool.tile([128, T, 1], F32, tag="negu")
                nc.vector.tensor_scalar_mul(negu[:], uu[:], -1.0)
                nc.vector.tensor_mul(ot[:, :, 12:13], negu[:], spoly[:])
                nc.vector.tensor_mul(ot[:, :, 13:16], nrm,
                                     cosu[:].to_broadcast([128, T, 3]))

                nc.vector.tensor_copy(ot[:, :, 16:17], scr8[:, :, 8:9])

                # ---- covariance ----
                q_v = ot[:, :, 13:16]
                pr = fpool.tile([128, T, 9], F32, tag="pr")
                nc.vector.tensor_mul(pr[:, :, 0:3], q_v, q_v)
                nc.vector.tensor_mul(pr[:, :, 3:4], q_v[:, :, 0:1], q_v[:, :, 1:2])
                nc.vector.tensor_mul(pr[:, :, 4:5], q_v[:, :, 0:1], q_v[:, :, 2:3])
                nc.vector.tensor_mul(pr[:, :, 5:6], q_v[:, :, 1:2], q_v[:, :, 2:3])
                nc.vector.tensor_mul(pr[:, :, 6:9], q_v,
                                     ot[:, :, 12:13].to_broadcast([128, T, 3]))

                xx, yy, zz = pr[:, :, 0:1], pr[:, :, 1:2], pr[:, :, 2:3]
                xy, xz, yz = pr[:, :, 3:4], pr[:, :, 4:5], pr[:, :, 5:6]
                rx, ry, rz = pr[:, :, 6:7], pr[:, :, 7:8], pr[:, :, 8:9]

                Rt = fpool.tile([128, T, 3, 3], F32, tag="Rt")
                t1 = fpool.tile([128, T, 1], F32, tag="t1")
                for i, (a, b) in enumerate([(yy, zz), (xx, zz), (xx, yy)]):
                    nc.vector.tensor_add(t1[:], a, b)
                    nc.vector.tensor_scalar(Rt[:, :, i, i:i + 1], t1[:], -1.0, 0.5,
                                            ALU.mult, ALU.add)
                nc.vector.tensor_sub(Rt[:, :, 0, 1:2], xy, rz)
                nc.vector.tensor_add(Rt[:, :, 0, 2:3], xz, ry)
                nc.vector.tensor_add(Rt[:, :, 1, 0:1], xy, rz)
                nc.vector.tensor_sub(Rt[:, :, 1, 2:3], yz, rx)
                nc.vector.tensor_sub(Rt[:, :, 2, 0:1], xz, ry)
                nc.vector.tensor_add(Rt[:, :, 2, 1:2], yz, rx)

                s2 = fpool.tile([128, T, 3], F32, tag="s2")
                nc.vector.tensor_scalar_mul(s2[:], scl, 2.0)
                L = fpool.tile([128, T, 3, 3], F32, tag="L")
                nc.vector.tensor_mul(
                    L[:], Rt[:], s2[:, :, None, :].to_broadcast([128, T, 3, 3]))

                lp = fpool.tile([128, T, 3], F32, tag="lp")
                for o, (i, k) in enumerate([(0, 0), (0, 1), (0, 2), (1, 1),
                                            (1, 2), (2, 2)]):
                    nc.vector.tensor_mul(lp[:], L[:, :, i, :], L[:, :, k, :])
                    nc.vector.reduce_sum(ot[:, :, 17 + o:18 + o], lp[:],
                                         axis=mybir.AxisListType.X)

            # ---- store ----
            nc.sync.dma_start(
                out_ap[base:base + TILE_N, :].rearrange("(p j) c -> p (j c)", p=128),
                ot[:].rearrange("p a b -> p (a b)"))

        for p in (ppool2, ppoolh, ppool, fpool, spool, wpool):
            p.release()

    nc.compile()
    return nc


def _prep_host(inputs):
    faces = np.ascontiguousarray(np.asarray(inputs["faces"], dtype=np.int32))
    verts = np.ascontiguousarray(np.asarray(inputs["vertices"], dtype=np.float32))
    f64 = lambda k: np.asarray(inputs[k], dtype=np.float64)

    wc = (f64("geo_w1")[:, 1:] @ f64("rw0")[9:, :]).astype(np.float32)
    rb0e = (f64("rb0") + f64("geo_b1")[1:] @ f64("rw0")[9:, :]).astype(np.float32)
    hw8 = np.concatenate([f64("rw4"), f64("sw"), f64("aw"),
                          np.zeros((DH, 1))], axis=1).astype(np.float32)
    hb10 = np.concatenate([f64("rb4"), f64("sb"), f64("ab"), [0.0],
                           f64("geo_b1")[:1], [0.0]]).astype(np.float32).reshape(1, 10)
    wo2 = np.concatenate([f64("geo_w1")[:, :1], np.zeros((DH, 1))],
                         axis=1).astype(np.float32)

    shared = {
        "verts": verts,
        "cam": np.asarray(inputs["camera_center"], dtype=np.float32).reshape(1, 3),
        "gw0": np.asarray(inputs["geo_w0"], dtype=np.float32),
        "gb0": np.asarray(inputs["geo_b0"], dtype=np.float32),
        "wc": wc,
        "rgeom": np.ascontiguousarray(np.asarray(inputs["rw0"], dtype=np.float32)[:9, :]),
        "rb0e": rb0e,
        "rw1": np.asarray(inputs["rw1"], dtype=np.float32),
        "rb1": np.asarray(inputs["rb1"], dtype=np.float32),
        "rw2": np.asarray(inputs["rw2"], dtype=np.float32),
        "rb2": np.asarray(inputs["rb2"], dtype=np.float32),
        "rw3": np.asarray(inputs["rw3"], dtype=np.float32),
        "rb3": np.asarray(inputs["rb3"], dtype=np.float32),
        "hw8": hw8,
        "hb10": hb10,
        "wo": wo2,
    }
    in_maps = []
    for c in range(N_CORES):
        fc = faces[c * F_CORE:(c + 1) * F_CORE]
        fc = np.concatenate([fc, np.zeros((F_PAD - F_CORE, 3), np.int32)], axis=0)
        in_maps.append({**shared, "faces": fc})
    return in_maps


def get_program():
    if "nc" not in _CACHE:
        _CACHE["nc"] = _build_program()
    return _CACHE["nc"]


def kernel(**inputs) -> np.ndarray:
    nc = get_program()
    in_maps = _prep_host(inputs)
    res = run_bass_kernel_spmd(nc, in_maps, core_ids=list(range(N_CORES)))
    out = np.concatenate([res.results[c]["out"][:F_CORE] for c in range(N_CORES)],
                         axis=0)
    return out
